# revision 1
# baseline (speedup 1.0000x reference)
"""Trainium2 Bass kernel for nn_DenoiserBlock (B=2, L=2048, D=1024, H=16, F=4096).

Sharding: 8 cores = 2 (batch) x 4 (query-slice of 512). Each core computes
K/V for the full sequence of its batch element (data redundancy instead of
collectives), attention + MLP for its 512-query slice. Host does the
(cheap) AdaLN modulation precompute, weight re-layout/casting, and final
concatenation of the 8 [512, 1024] output slices.

Device dataflow (per core, "T" = feature-major / transposed layout):
  A: LN1+AdaLN over x[b] -> h, PE-transpose -> hT (bf16); same for the
     512 residual rows -> hresT
  B: kT = Wk^T hT, v = hT^T Wv (with an appended ones-column per head for
     softmax denominators), qT = Wq_s^T hresT
  C: per head: sT = kT_h^T qT_h (K=64 matmul), DVE fuses the torus bias,
     ACT exp, attn@v accumulated over 16 key tiles; row 64 of the psum is
     the softmax denominator; normalize via reciprocal + PE broadcast
  D: out2 = OUT^T^T Wout + x_res, LN2, transpose -> h2T
  E: aT[f-tile] = gelu(W1^T h2T + b1) for 32 f-tiles, then
     y = aT^T W2 + (x2 + b2)
"""

import sys

sys.path.insert(0, "/opt/trn_rl_repo")

import numpy as np
import ml_dtypes

import concourse.bacc as bacc
import concourse.mybir as mybir
from concourse import tile, masks
from concourse.bass_utils import run_bass_kernel_spmd

F32 = mybir.dt.float32
BF16 = mybir.dt.bfloat16
F32R = mybir.dt.float32r
AX = mybir.AxisListType
OP = mybir.AluOpType
ACT = mybir.ActivationFunctionType

B, L, D, H, F = 2, 2048, 1024, 16, 4096
HD = D // H          # 64
QS = 512             # queries per core
NC_PER_B = 4
EPS = 1e-5

_CACHED = {}


def _build(shared_mask=True):
    nc = bacc.Bacc("TRN2", target_bir_lowering=False, debug=False, num_devices=8)

    d_x = nc.dram_tensor("x_full", [L, D], F32, kind="ExternalInput")
    d_xres = nc.dram_tensor("x_res", [QS, D], F32, kind="ExternalInput")
    if shared_mask:
        d_expm = nc.dram_tensor("expm", [L, QS], BF16, kind="ExternalInput")
    else:
        d_expm = nc.dram_tensor("expm", [H, L, QS], BF16, kind="ExternalInput")
    d_wq = nc.dram_tensor("wq", [D, D], BF16, kind="ExternalInput")
    d_wk = nc.dram_tensor("wk", [D, D], BF16, kind="ExternalInput")
    d_wv = nc.dram_tensor("wv", [D, D], BF16, kind="ExternalInput")
    d_wout = nc.dram_tensor("wout", [D, D], BF16, kind="ExternalInput")
    d_w1t = nc.dram_tensor("w1t", [32, 8, 128, 128], BF16, kind="ExternalInput")
    d_w2 = nc.dram_tensor("w2", [F, D], BF16, kind="ExternalInput")
    d_bias2r = nc.dram_tensor("bias2r", [128, D], F32, kind="ExternalInput")
    d_biask = nc.dram_tensor("biask", [128, 8], F32, kind="ExternalInput")
    d_biasq = nc.dram_tensor("biasq", [128, 8], F32, kind="ExternalInput")
    d_bvrep = nc.dram_tensor("bvrep", [128, D], F32, kind="ExternalInput")
    d_b1sb = nc.dram_tensor("b1sb", [128, 32], F32, kind="ExternalInput")
    d_y = nc.dram_tensor("y", [QS, D], F32, kind="ExternalOutput")

    NLT = L // 128
    NDT = D // 128
    NQT = QS // 128
    NFT = F // 128

    with tile.TileContext(nc) as tc:
        with (
            tc.tile_pool(name="const", bufs=1) as cpool,
            tc.tile_pool(name="mid", bufs=1) as mpool,
            tc.tile_pool(name="psum", bufs=1, space="PSUM") as pspool,
        ):
            b1sb = cpool.tile([128, 32], F32, tag="b1sb")
            ident = cpool.tile([128, 128], BF16, tag="ident")
            epsc = cpool.tile([128, 1], F32, tag="epsc")
            biask = cpool.tile([128, 8], F32, tag="biask")
            biasq = cpool.tile([128, 8], F32, tag="biasq")
            bvrep = cpool.tile([128, D], F32, tag="bvrep")
            nc.sync.dma_start(b1sb[:], d_b1sb[:, :])
            nc.sync.dma_start(biask[:], d_biask[:, :])
            nc.sync.dma_start(biasq[:], d_biasq[:, :])
            nc.sync.dma_start(bvrep[:], d_bvrep[:, :])
            masks.make_identity(nc, ident[:])
            nc.vector.memset(epsc[:], EPS)

            outT = [mpool.tile([128, QS], BF16, tag=f"outT{i}", name=f"outT{i}")
                    for i in range(NDT)]
            x2 = [mpool.tile([128, D], F32, tag=f"x2{i}", name=f"x2{i}")
                  for i in range(NQT)]
            h2T = [mpool.tile([128, QS], BF16, tag=f"h2T{i}", name=f"h2T{i}")
                   for i in range(NDT)]

            def layer_norm_tile(pool, pspool, xt, hT_tiles, col0):
                """Normalize one [128, D] tile (no gain/bias - folded into the
                consuming weights host-side) -> bf16 transposed blocks into
                hT_tiles[j][:, col0:col0+128]."""
                s1 = pool.tile([128, 1], F32, tag="lns", name="s1", bufs=21)
                s2 = pool.tile([128, 1], F32, tag="lns", name="s2", bufs=21)
                mu = pool.tile([128, 1], F32, tag="lns", name="mu", bufs=21)
                msq = pool.tile([128, 1], F32, tag="lns", name="msq", bufs=21)
                var = pool.tile([128, 1], F32, tag="lns", name="var", bufs=21)
                std = pool.tile([128, 1], F32, tag="lns", name="std", bufs=21)
                rstd = pool.tile([128, 1], F32, tag="lns", name="rstd", bufs=21)
                sq = pool.tile([128, D], F32, tag="xc", name="sq")
                hb = pool.tile([128, D], BF16, tag="hb", name="hb")
                nc.vector.tensor_reduce(s1[:], xt[:], axis=AX.X, op=OP.add)
                nc.scalar.activation(sq[:], xt[:], ACT.Square, accum_out=s2[:])
                nc.scalar.mul(mu[:], s1[:], 1.0 / D)
                nc.vector.tensor_tensor(msq[:], mu[:], mu[:], op=OP.mult)
                nc.vector.scalar_tensor_tensor(
                    var[:], s2[:], 1.0 / D, msq[:], op0=OP.mult, op1=OP.subtract)
                nc.scalar.activation(std[:], var[:], ACT.Sqrt, bias=epsc[:])
                nc.vector.reciprocal(rstd[:], std[:])
                nc.vector.tensor_scalar(hb[:], xt[:], mu[:], rstd[:],
                                        op0=OP.subtract, op1=OP.mult)
                for j in range(NDT):
                    pt = pspool.tile([128, 128], BF16, tag="trp", name="trp", bufs=2)
                    nc.tensor.transpose(pt[:], hb[:, j * 128:(j + 1) * 128], ident[:])
                    if j % 2 == 0:
                        nc.scalar.copy(hT_tiles[j][:, col0:col0 + 128], pt[:])
                    else:
                        nc.vector.tensor_copy(hT_tiles[j][:, col0:col0 + 128], pt[:])

            with tc.tile_pool(name="attn", bufs=1) as atpool:
                kT = [atpool.tile([128, L], BF16, tag=f"kT{i}", name=f"kT{i}")
                      for i in range(NDT)]
                vv = [atpool.tile([128, H * (HD + 1)], BF16, tag=f"v{i}", name=f"v{i}")
                      for i in range(NLT)]
                qT = [atpool.tile([128, QS], BF16, tag=f"qT{i}", name=f"qT{i}")
                      for i in range(NDT)]

                # ---- Phase A ----
                with tc.tile_pool(name="hTp", bufs=1) as hpool:
                    hT = [hpool.tile([128, L], BF16, tag=f"hT{i}", name=f"hT{i}")
                          for i in range(NDT)]
                    hresT = [hpool.tile([128, QS], BF16, tag=f"hrT{i}", name=f"hrT{i}")
                             for i in range(NDT)]
                    with tc.tile_pool(name="phA", bufs=5) as apool:
                        for lt in range(NLT):
                            xt = apool.tile([128, D], F32, tag="xt", name="xt", bufs=4)
                            nc.sync.dma_start(xt[:], d_x[lt * 128:(lt + 1) * 128, :])
                            layer_norm_tile(apool, pspool, xt, hT, lt * 128)
                        for rt in range(NQT):
                            xt = apool.tile([128, D], F32, tag="xt", name="xt", bufs=4)
                            nc.sync.dma_start(xt[:], d_xres[rt * 128:(rt + 1) * 128, :])
                            layer_norm_tile(apool, pspool, xt, hresT, rt * 128)

                    # ---- Phase B ----
                    with tc.tile_pool(name="wtsQ", bufs=1) as wqpool:
                        wq = [wqpool.tile([128, D], BF16, tag=f"wq{i}", name=f"wq{i}")
                              for i in range(NDT)]
                        for i in range(NDT):
                            nc.sync.dma_start(wq[i][:], d_wq[i * 128:(i + 1) * 128, :])
                        for i in range(NDT):
                            pq = pspool.tile([128, 512], F32, tag="mm", name="pq", bufs=4)
                            for dt_ in range(NDT):
                                nc.tensor.matmul(
                                    pq[:], wq[dt_][:, i * 128:(i + 1) * 128],
                                    hresT[dt_][:],
                                    start=(dt_ == 0), stop=(dt_ == NDT - 1))
                            nc.vector.tensor_scalar(qT[i][:], pq[:], biasq[:, i:i + 1],
                                                    None, op0=OP.add)

                    with tc.tile_pool(name="wtsK", bufs=1) as wkpool:
                        wk = [wkpool.tile([128, D], BF16, tag=f"wk{i}", name=f"wk{i}")
                              for i in range(NDT)]
                        for i in range(NDT):
                            nc.sync.dma_start(wk[i][:], d_wk[i * 128:(i + 1) * 128, :])
                        for i in range(NDT):
                            for ncol in range(L // 512):
                                pk = pspool.tile([128, 512], F32, tag="mm", name="pk", bufs=4)
                                for dt_ in range(NDT):
                                    nc.tensor.matmul(
                                        pk[:], wk[dt_][:, i * 128:(i + 1) * 128],
                                        hT[dt_][:, ncol * 512:(ncol + 1) * 512],
                                        start=(dt_ == 0), stop=(dt_ == NDT - 1))
                                nc.vector.tensor_scalar(
                                    kT[i][:, ncol * 512:(ncol + 1) * 512], pk[:],
                                    biask[:, i:i + 1], None, op0=OP.add)

                    with tc.tile_pool(name="wtsV", bufs=1) as wvpool:
                        wv = [wvpool.tile([128, D], BF16, tag=f"wv{i}", name=f"wv{i}")
                              for i in range(NDT)]
                        for i in range(NDT):
                            nc.sync.dma_start(wv[i][:], d_wv[i * 128:(i + 1) * 128, :])
                        for lt in range(NLT):
                            v3 = vv[lt][:].rearrange("p (h c) -> p h c", c=HD + 1)
                            for half in range(2):
                                pv = pspool.tile([128, 512], F32, tag="mm", name="pv", bufs=4)
                                for dt_ in range(NDT):
                                    nc.tensor.matmul(
                                        pv[:], hT[dt_][:, lt * 128:(lt + 1) * 128],
                                        wv[dt_][:, half * 512:(half + 1) * 512],
                                        start=(dt_ == 0), stop=(dt_ == NDT - 1))
                                nc.vector.tensor_tensor(
                                    v3[:, half * 8:(half + 1) * 8, 0:HD], pv[:],
                                    bvrep[:, half * 512:(half + 1) * 512], op=OP.add)
                            nc.vector.memset(v3[:, :, HD:HD + 1], 1.0)

                # ---- Phase C ----
                with (
                    tc.tile_pool(name="phC", bufs=8) as cwork,
                    tc.tile_pool(name="mres", bufs=1) as mpool_c,
                ):
                    mres = None
                    if shared_mask:
                        mres = [mpool_c.tile([128, QS], BF16, tag=f"mr{i}",
                                             name=f"mr{i}") for i in range(NLT)]
                        for kt in range(NLT):
                            nc.sync.dma_start(
                                mres[kt][:], d_expm[kt * 128:(kt + 1) * 128, :])
                    for hp in range(H // 2):
                        ht = hp
                        pos = [pspool.tile([65, 512], F32, tag="acc",
                                           name=f"po{par}", bufs=2) for par in range(2)]
                        for kt in range(NLT):
                            for par in range(2):
                                h, ho = 2 * hp + par, par * 64
                                if shared_mask:
                                    mt = mres[kt]
                                else:
                                    mt = cwork.tile([128, 512], BF16, tag="mt",
                                                    name="mt")
                                    nc.sync.dma_start(
                                        mt[:], d_expm[h, kt * 128:(kt + 1) * 128, :])
                                ps = pspool.tile([128, 512], F32, tag="mm",
                                                 name="ps", bufs=4)
                                nc.tensor.matmul(
                                    ps[:], kT[ht][ho:ho + 64, kt * 128:(kt + 1) * 128],
                                    qT[ht][ho:ho + 64, :], start=True, stop=True)
                                pb = cwork.tile([128, 512], BF16, tag="pb", name="pb")
                                nc.scalar.activation(pb[:], ps[:], ACT.Exp)
                                pm = cwork.tile([128, 512], BF16, tag="pm", name="pm")
                                nc.vector.tensor_tensor(pm[:], pb[:], mt[:],
                                                        op=OP.mult)
                                v3 = vv[kt][:].rearrange("p (h c) -> p h c", c=HD + 1)
                                nc.tensor.matmul(
                                    pos[par][:], v3[:, h, :], pm[:],
                                    start=(kt == 0), stop=(kt == NLT - 1))
                        for par in range(2):
                            ho = par * 64
                            rsum = cwork.tile([1, 512], F32, tag="recip",
                                              name="rsum")
                            nc.vector.tensor_scalar(rsum[:], pos[par][64:65, :],
                                                    1e-30, None, op0=OP.add)
                            recip = cwork.tile([1, 512], F32, tag="recip",
                                               name="recip")
                            nc.vector.reciprocal(recip[:], rsum[:])
                            rbs = cwork.tile([64, 512], F32, tag="rbs", name="rbs")
                            nc.gpsimd.partition_broadcast(rbs[:], recip[:])
                            nc.vector.tensor_tensor(
                                outT[ht][ho:ho + 64, :], pos[par][0:64, :], rbs[:],
                                op=OP.mult)

                # ---- Phase D ----
                with (
                    tc.tile_pool(name="phD", bufs=6) as dwork,
                    tc.tile_pool(name="phD_w", bufs=1) as dwpool,
                ):
                    bias2r = dwpool.tile([128, D], F32, tag="bias2r")
                    nc.sync.dma_start(bias2r[:], d_bias2r[:, :])
                    wo = [dwpool.tile([128, D], BF16, tag=f"wo{i}", name=f"wo{i}")
                          for i in range(NDT)]
                    for i in range(NDT):
                        nc.sync.dma_start(wo[i][:], d_wout[i * 128:(i + 1) * 128, :])
                    xr = [dwpool.tile([128, D], F32, tag=f"xr{i}", name=f"xr{i}")
                          for i in range(NQT)]
                    for i in range(NQT):
                        nc.sync.dma_start(xr[i][:], d_xres[i * 128:(i + 1) * 128, :])
                    for qt in range(NQT):
                        for half in range(2):
                            p2 = pspool.tile([128, 512], F32, tag="mm", name="p2", bufs=4)
                            for dt_ in range(NDT):
                                nc.tensor.matmul(
                                    p2[:], outT[dt_][:, qt * 128:(qt + 1) * 128],
                                    wo[dt_][:, half * 512:(half + 1) * 512],
                                    start=(dt_ == 0), stop=(dt_ == NDT - 1))
                            nc.vector.tensor_tensor(
                                x2[qt][:, half * 512:(half + 1) * 512], p2[:],
                                xr[qt][:, half * 512:(half + 1) * 512], op=OP.add)
                        layer_norm_tile(dwork, pspool, x2[qt], h2T, qt * 128)
                        nc.vector.tensor_tensor(x2[qt][:], x2[qt][:], bias2r[:],
                                                op=OP.add)

            # ---- Phase E ----
            with (
                tc.tile_pool(name="phE_a", bufs=1) as e_apool,
                tc.tile_pool(name="phE_w", bufs=4) as e_wpool,
                tc.tile_pool(name="phE_w2", bufs=1) as e_w2pool,
                tc.tile_pool(name="phE", bufs=3) as e_work,
            ):
                aT = [e_apool.tile([128, QS], BF16, tag=f"aT{i}", name=f"aT{i}")
                      for i in range(NFT)]
                w2sb = [e_w2pool.tile([128, D], BF16, tag=f"w2_{i}", name=f"w2_{i}")
                        for i in range(NFT)]
                for ft in range(NFT):
                    nc.sync.dma_start(w2sb[ft][:], d_w2[ft * 128:(ft + 1) * 128, :])
                for ft in range(NFT):
                    w1b = e_wpool.tile([128, D], BF16, tag="w1b", name="w1b")
                    nc.sync.dma_start(
                        w1b[:].rearrange("p (d c) -> p d c", c=128),
                        d_w1t[ft].rearrange("d r c -> r d c"))
                    pa = pspool.tile([128, 512], F32, tag="mm", name="pa", bufs=4)
                    for dt_ in range(NDT):
                        nc.tensor.matmul(
                            pa[:], w1b[:, dt_ * 128:(dt_ + 1) * 128], h2T[dt_][:],
                            start=(dt_ == 0), stop=(dt_ == NDT - 1))
                    nc.scalar.activation(aT[ft][:], pa[:], ACT.Gelu_apprx_tanh,
                                         bias=b1sb[:, ft:ft + 1])
                for qt in range(NQT):
                    ysb = e_work.tile([128, D], F32, tag="ysb", name="ysb")
                    for half in range(2):
                        p3 = pspool.tile([128, 512], F32, tag="acc", name="p3", bufs=2)
                        for ft in range(NFT):
                            nc.tensor.matmul(
                                p3[:], aT[ft][:, qt * 128:(qt + 1) * 128],
                                w2sb[ft][:, half * 512:(half + 1) * 512],
                                start=(ft == 0), stop=(ft == NFT - 1))
                        nc.vector.tensor_tensor(
                            ysb[:, half * 512:(half + 1) * 512], p3[:],
                            x2[qt][:, half * 512:(half + 1) * 512], op=OP.add)
                    nc.sync.dma_start(d_y[qt * 128:(qt + 1) * 128, :], ysb[:])

    nc.compile()
    return nc


def _gelu_tanh(x):
    x = x.astype(np.float64)
    return 0.5 * x * (1.0 + np.tanh(np.sqrt(2.0 / np.pi) * (x + 0.044715 * x ** 3)))


def kernel(x, torus_dist, time_emb, mask, ln1_g, ln1_b, Wqkv, Wout,
           torus_scale, ln2_g, ln2_b, W1, b1, W2, b2, Wt, bt):
    x = np.asarray(x, np.float32)
    torus_dist = np.asarray(torus_dist, np.float32)
    time_emb = np.asarray(time_emb, np.float32)
    mask = np.asarray(mask)
    Wqkv = np.asarray(Wqkv, np.float32)

    sc_arr = np.asarray(torus_scale, np.float32)
    shared = bool(np.all(sc_arr == sc_arr[0]))
    key = f"nc_{shared}"
    if key not in _CACHED:
        _CACHED[key] = _build(shared_mask=shared)
    nc = _CACHED[key]

    bf = lambda a: np.ascontiguousarray(a).astype(ml_dtypes.bfloat16)
    rep = lambda v: np.ascontiguousarray(
        np.tile(np.asarray(v, np.float32)[None, :], (128, 1)))

    tp = (_gelu_tanh(time_emb) @ np.asarray(Wt, np.float64)
          + np.asarray(bt, np.float64))          # [B, 2D]
    scale, shift = tp[:, :D], tp[:, D:]
    g_eff = (np.asarray(ln1_g, np.float64)[None, :] * (1.0 + scale)).astype(np.float32)
    b_eff = (np.asarray(ln1_b, np.float64)[None, :] * (1.0 + scale) + shift).astype(np.float32)

    Wq_r = np.asarray(Wqkv[:, 0:D], np.float64) / np.sqrt(64.0)
    Wk_r = np.asarray(Wqkv[:, D:2 * D], np.float64)
    Wv_r = np.asarray(Wqkv[:, 2 * D:3 * D], np.float64)
    W1_r = np.asarray(W1, np.float64)
    g2 = np.asarray(ln2_g, np.float64)
    b2ln = np.asarray(ln2_b, np.float64)
    w1t_g = (g2[:, None] * W1_r).astype(np.float32)
    w1t = bf(w1t_g.reshape(8, 128, 32, 128).transpose(2, 0, 1, 3))
    b1sb_eff = (np.asarray(b1, np.float64) + b2ln @ W1_r).astype(np.float32)
    b1sb = np.ascontiguousarray(b1sb_eff.reshape(32, 128).T)
    w2 = bf(W2)
    wout = bf(Wout)
    bias2r = rep(b2)

    in_maps = []
    for c in range(8):
        b_, qs_ = c // NC_PER_B, c % NC_PER_B
        rows = slice(qs_ * QS, (qs_ + 1) * QS)
        km = np.where(mask[b_], 0.0, -88.0).astype(np.float32)      # [L]
        torT = torus_dist[0, rows, :].T.astype(np.float32)           # [L, QS]
        if shared:
            expm = np.exp(km[:, None] - sc_arr[0] * torT).astype(ml_dtypes.bfloat16)
        else:
            expm = np.exp(km[None, :, None] - sc_arr[:, None, None]
                          * torT[None, :, :]).astype(ml_dtypes.bfloat16)
        ge = g_eff[b_].astype(np.float64)
        be = b_eff[b_].astype(np.float64)
        wq_b = bf((ge[:, None] * Wq_r).astype(np.float32))
        wk_b = bf((ge[:, None] * Wk_r).astype(np.float32))
        wv_b = bf((ge[:, None] * Wv_r).astype(np.float32))
        bk = (be @ Wk_r).astype(np.float32)
        bq = (be @ Wq_r).astype(np.float32)
        bv = (be @ Wv_r).astype(np.float32)
        in_maps.append({
            "x_full": x[b_],
            "x_res": np.ascontiguousarray(x[b_, rows]),
            "expm": expm,
            "wq": wq_b, "wk": wk_b, "wv": wv_b, "wout": wout,
            "w1t": w1t, "w2": w2,
            "biask": np.ascontiguousarray(bk.reshape(8, 128).T),
            "biasq": np.ascontiguousarray(bq.reshape(8, 128).T),
            "bvrep": rep(bv),
            "bias2r": bias2r, "b1sb": b1sb,
        })

    import os
    trace = bool(int(os.environ.get("DENOISER_TRACE", "0")))
    res = run_bass_kernel_spmd(nc, in_maps, core_ids=list(range(8)), trace=trace)
    _CACHED["last_results"] = res

    out = np.empty((B, L, D), np.float32)
    for c in range(8):
        b_, qs_ = c // NC_PER_B, c % NC_PER_B
        out[b_, qs_ * QS:(qs_ + 1) * QS, :] = res.results[c]["y"]
    return out



# revision 21
# speedup vs baseline: 1.4841x; 1.4841x over previous
"""Trainium2 Bass kernel for nn_DenoiserBlock (B=2, L=2048, D=1024, H=16, F=4096).

Sharding: 8 cores = 2 (batch) x 4 (query-slice of 512). Each core computes
K/V for the full sequence of its batch element (no collectives), attention +
MLP for its 512-query slice. Host does AdaLN precompute, weight re-layout and
fp8 quantization, and final concatenation of the 8 [512, 1024] output slices.

Device dataflow (per core):
  A: LN1 over x[b] (bf16, pipelined groups of 4 row tiles) -> PE-transpose ->
     hT (fp8 DoubleRow pair layout); same for the 512 query rows -> hqT
  B: fp8 DoubleRow projections -> qT (fp8, zero-padded pair layout),
     kT (fp8), v (fp8, pair layout, appended ones column per head)
  C: per head: scores = DR(kT,qT) + DR(sc_h*I, -torus) accumulated in psum;
     wide exp(s - 5) (ACT) -> pm fp8; attn@v via fp8 DR over key-tile pairs
     (denominator in column 64); normalize*16, PE-transpose -> outT (fp8)
  D: x2 = DR(outT, Wout)/512 + xres; LN2 -> h2T (fp8) + dh2T (fp8 residual)
  E: aT = 8*gelu(DR(w1, h2T) + DR(dw1, h2T) + DR(w1, dh2T) + b1) (fp8);
     y = (DR(aT, w2) + DR(aT, dw2))/(8*SW) + x2   (b2 rides a w2 slot)

Algebraic folds (host): k-bias dropped (softmax shift-invariance per query),
LN gains folded into weights, mask+torus bias folded into an fp8 log-bias
tensor injected into the scores psum by an identity matmul, exp shifted by -5
(softmax-invariant) to keep fp8 probabilities in range, W1/W2 carry fp8
residual-compensation slots, b2 rides an extra W2 contraction slot.
"""

import sys

sys.path.insert(0, "/opt/trn_rl_repo")

import numpy as np
import ml_dtypes

import concourse.bacc as bacc
import concourse.mybir as mybir
from concourse import tile, masks
from concourse.bass_utils import run_bass_kernel_spmd

F32 = mybir.dt.float32
BF16 = mybir.dt.bfloat16
FP8 = mybir.dt.float8e4
AX = mybir.AxisListType
OP = mybir.AluOpType
ACT = mybir.ActivationFunctionType
PM = mybir.MatmulPerfMode

B, L, D, H, F = 2, 2048, 1024, 16, 4096
HD = D // H          # 64
QS = 512             # queries per core
NC_PER_B = 4
NLT = L // 128       # 16
NDT = D // 128       # 8
NQT = QS // 128      # 4
NFT = F // 128       # 32
EPS = 1e-5
SW = 32.0            # fp8 weight upscale
SA = 16.0            # attn-out upscale
SG = 8.0             # gelu-out upscale
ESH = -5.0           # softmax exp shift

_CACHED = {}


def _build(allmask=True):
    nc = bacc.Bacc("TRN2", target_bir_lowering=False, debug=False, num_devices=8)

    d_xt = nc.dram_tensor("xt", [128, NLT, D], BF16, kind="ExternalInput")
    d_xres = nc.dram_tensor("xres", [128, NQT, D], F32, kind="ExternalInput")
    d_tor = nc.dram_tensor("tor", [128, NLT + 1, QS], FP8, kind="ExternalInput")
    d_iddr = nc.dram_tensor("iddr", [128, H, 2, 128], FP8, kind="ExternalInput")
    d_wq = nc.dram_tensor("wq", [128, 4, 2, D], FP8, kind="ExternalInput")
    d_wk = nc.dram_tensor("wk", [128, 4, 2, D], FP8, kind="ExternalInput")
    d_wv = nc.dram_tensor("wv", [128, 4, 2, D], FP8, kind="ExternalInput")
    d_wo = nc.dram_tensor("wo", [128, 4, 2, D], FP8, kind="ExternalInput")
    d_w1 = nc.dram_tensor("w1", [128, 8, 2, F], FP8, kind="ExternalInput")
    d_w2 = nc.dram_tensor("w2", [128, 34, 2, D], FP8, kind="ExternalInput")
    d_biasq = nc.dram_tensor("biasq", [128, NDT], F32, kind="ExternalInput")
    d_b1sb = nc.dram_tensor("b1sb", [128, NFT], F32, kind="ExternalInput")
    d_km = nc.dram_tensor("km", [128, NLT], F32, kind="ExternalInput")
    d_bvrep = nc.dram_tensor("bvrep", [128, D], F32, kind="ExternalInput")
    d_y = nc.dram_tensor("y", [128, NQT, D], F32, kind="ExternalOutput")

    with tile.TileContext(nc) as tc:
        with (
            tc.tile_pool(name="const", bufs=1) as cpool,
            tc.tile_pool(name="mid", bufs=1) as mpool,
            tc.tile_pool(name="psum", bufs=1, space="PSUM") as pspool,
        ):
            identb = cpool.tile([128, 128], BF16, tag="identb")
            epsc = cpool.tile([128, 1], F32, tag="epsc")
            eshc = cpool.tile([128, 1], F32, tag="eshc")
            biasq = cpool.tile([128, NDT], F32, tag="biasq")
            b1sb = cpool.tile([128, NFT], F32, tag="b1sb")
            km = cpool.tile([128, NLT], F32, tag="km")
            masks.make_identity(nc, identb[:])
            nc.vector.memset(epsc[:], EPS)
            nc.vector.memset(eshc[:], ESH)
            nc.sync.dma_start(biasq[:], d_biasq[:])
            nc.sync.dma_start(b1sb[:], d_b1sb[:])
            nc.sync.dma_start(km[:], d_km[:])

            outT = mpool.tile([128, NDT, QS], FP8, tag="outT", name="outT")
            x2 = mpool.tile([128, NQT, D], F32, tag="x2", name="x2")
            h2T = mpool.tile([128, NDT, QS], FP8, tag="h2T", name="h2T")
            dh2T = mpool.tile([128, NDT, QS], FP8, tag="dh2T", name="dh2T")

            def ln_stats(pool, xt_ap, s1c, s2c):
                sq = pool.tile([128, D], BF16, tag="sq", name="sq", bufs=3)
                nc.vector.tensor_reduce(s1c, xt_ap, axis=AX.X, op=OP.add)
                nc.scalar.activation(sq[:], xt_ap, ACT.Square, accum_out=s2c)

            def ln_finalize(pool, s1a, s2a, n, rstd_a, nmr_a, tag):
                mu = pool.tile([128, n], F32, tag=tag, name="mu", bufs=10)
                ms = pool.tile([128, n], F32, tag=tag, name="ms", bufs=10)
                var = pool.tile([128, n], F32, tag=tag, name="var", bufs=10)
                std = pool.tile([128, n], F32, tag=tag, name="std", bufs=10)
                nc.vector.tensor_scalar(mu[:], s1a, 1.0 / D, None, op0=OP.mult)
                nc.vector.tensor_tensor(ms[:], mu[:], mu[:], op=OP.mult)
                nc.vector.scalar_tensor_tensor(
                    var[:], s2a, 1.0 / D, ms[:], op0=OP.mult, op1=OP.subtract)
                nc.scalar.activation(std[:], var[:], ACT.Sqrt, bias=epsc[:])
                nc.vector.reciprocal(rstd_a, std[:])
                nc.vector.scalar_tensor_tensor(
                    nmr_a, mu[:], -1.0, rstd_a, op0=OP.mult, op1=OP.mult)

            def ln_norm_transpose(pool, xt_ap, rstd_c, nmr_c, dstT, col0,
                                  ddstT=None):
                hb = pool.tile([128, D], BF16, tag="hb", name="hb", bufs=3)
                nc.vector.tensor_scalar(hb[:], xt_ap, rstd_c, nmr_c,
                                        op0=OP.mult, op1=OP.add)
                for half in range(2):
                    pst = pspool.tile([128, 512], BF16, tag="trp", name="pst",
                                      bufs=2)
                    for j in range(4):
                        dt_ = half * 4 + j
                        nc.tensor.transpose(
                            pst[:, j * 128:(j + 1) * 128],
                            hb[:, dt_ * 128:(dt_ + 1) * 128], identb[:])
                    p3 = pst[:].rearrange("p (a b) -> p a b", b=128)
                    dst = dstT[:, half * 4:half * 4 + 4, col0:col0 + 128]
                    nc.vector.tensor_copy(dst, p3)
                    if ddstT is not None:
                        nc.vector.tensor_tensor(
                            ddstT[:, half * 4:half * 4 + 4, col0:col0 + 128],
                            p3, dst, op=OP.subtract)

            with tc.tile_pool(name="attn", bufs=1) as atpool:
                kT = atpool.tile([128, NDT + 1, L], FP8, tag="kT", name="kT")
                qT = atpool.tile([128, NDT, 2, QS], FP8, tag="qT", name="qT")
                vv = atpool.tile([128, NLT // 2, 2, H, HD + 1], FP8,
                                 tag="vv", name="vv")
                tor = atpool.tile([128, NLT + 1, QS], FP8, tag="tor", name="tor")
                iddr = atpool.tile([128, H, 2, 128], FP8, tag="iddr",
                                   name="iddr")
                nc.sync.dma_start(tor[:], d_tor[:])
                nc.sync.dma_start(iddr[:], d_iddr[:])
                nc.gpsimd.memset(kT[:, NDT, :], 0.0)
                nc.gpsimd.memset(qT[:, :, 1, :], 0.0)
                nc.gpsimd.memset(vv[:, :, :, :, HD], 1.0)

                # ---- Phase A ----
                with tc.tile_pool(name="hTp", bufs=1) as hpool:
                    hT = hpool.tile([128, NDT, L], FP8, tag="hT", name="hT")
                    hqT = hpool.tile([128, NDT, QS], FP8, tag="hqT", name="hqT")
                    with tc.tile_pool(name="phA", bufs=1) as apool:
                        xt = apool.tile([128, NLT, D], BF16, tag="xt", name="xt")
                        xres = apool.tile([128, NQT, D], F32, tag="xres",
                                          name="xres")
                        nc.sync.dma_start(xres[:], d_xres[:])
                        for c in range(4):
                            nc.sync.dma_start(xt[:, c * 4:(c + 1) * 4, :],
                                              d_xt[:, c * 4:(c + 1) * 4, :])
                        s1a = apool.tile([128, NLT], F32, tag="s1a", name="s1a")
                        s2a = apool.tile([128, NLT], F32, tag="s2a", name="s2a")
                        rstd = apool.tile([128, NLT], F32, tag="rstd",
                                          name="rstd")
                        nmr = apool.tile([128, NLT], F32, tag="nmr", name="nmr")
                        for g in range(4):
                            s_ = slice(4 * g, 4 * g + 4)
                            for lt in range(4 * g, 4 * g + 4):
                                ln_stats(apool, xt[:, lt, :],
                                         s1a[:, lt:lt + 1], s2a[:, lt:lt + 1])
                            ln_finalize(apool, s1a[:, s_], s2a[:, s_], 4,
                                        rstd[:, s_], nmr[:, s_], "lnfA")
                            for lt in range(4 * g, 4 * g + 4):
                                ln_norm_transpose(apool, xt[:, lt, :],
                                                  rstd[:, lt:lt + 1],
                                                  nmr[:, lt:lt + 1], hT,
                                                  lt * 128)
                        s1q = apool.tile([128, NQT], F32, tag="s1q", name="s1q")
                        s2q = apool.tile([128, NQT], F32, tag="s2q", name="s2q")
                        rstdq = apool.tile([128, NQT], F32, tag="rstdq",
                                           name="rstdq")
                        nmrq = apool.tile([128, NQT], F32, tag="nmrq",
                                          name="nmrq")
                        for qt in range(NQT):
                            ln_stats(apool, xres[:, qt, :], s1q[:, qt:qt + 1],
                                     s2q[:, qt:qt + 1])
                        ln_finalize(apool, s1q[:], s2q[:], NQT, rstdq[:],
                                    nmrq[:], "lnfA")
                        for qt in range(NQT):
                            ln_norm_transpose(apool, xres[:, qt, :],
                                              rstdq[:, qt:qt + 1],
                                              nmrq[:, qt:qt + 1], hqT,
                                              qt * 128)

                    # ---- Phase B ----
                    with tc.tile_pool(name="wtsB", bufs=1) as wbpool:
                        wq = wbpool.tile([128, 4, 2, D], FP8, tag="wq",
                                         name="wq")
                        wk = wbpool.tile([128, 4, 2, D], FP8, tag="wk",
                                         name="wk")
                        wv = wbpool.tile([128, 4, 2, D], FP8, tag="wv",
                                         name="wv")
                        bvrep = wbpool.tile([128, D], F32, tag="bvrep",
                                            name="bvrep")
                        nc.sync.dma_start(wq[:], d_wq[:])
                        nc.sync.dma_start(wk[:], d_wk[:])
                        nc.sync.dma_start(wv[:], d_wv[:])
                        nc.sync.dma_start(bvrep[:], d_bvrep[:])

                        for i in range(NDT):
                            pq = pspool.tile([128, 1024], F32, tag="mm",
                                             name="pq", bufs=2)
                            for j in range(4):
                                nc.tensor.matmul(
                                    pq[:, 0:512],
                                    wq[:, j, :, i * 128:(i + 1) * 128],
                                    hqT[:, 2 * j:2 * j + 2, :],
                                    start=(j == 0), stop=(j == 3),
                                    perf_mode=PM.DoubleRow)
                            nc.vector.tensor_scalar(
                                qT[:, i, 0, :], pq[:, 0:512], 1.0 / SW,
                                biasq[:, i:i + 1], op0=OP.mult, op1=OP.add)
                        for i in range(NDT):
                            for cc in range(2):
                                pk = pspool.tile([128, 1024], F32, tag="mm",
                                                 name="pk", bufs=2)
                                for half in range(2):
                                    c0 = cc * 1024 + half * 512
                                    for j in range(4):
                                        nc.tensor.matmul(
                                            pk[:, half * 512:(half + 1) * 512],
                                            wk[:, j, :, i * 128:(i + 1) * 128],
                                            hT[:, 2 * j:2 * j + 2, c0:c0 + 512],
                                            start=(j == 0), stop=(j == 3),
                                            perf_mode=PM.DoubleRow)
                                nc.vector.tensor_scalar(
                                    kT[:, i, cc * 1024:(cc + 1) * 1024], pk[:],
                                    1.0 / SW, None, op0=OP.mult)
                        for lt in range(NLT):
                            pv = pspool.tile([128, 1024], F32, tag="mm",
                                             name="pv", bufs=2)
                            for half in range(2):
                                for j in range(4):
                                    nc.tensor.matmul(
                                        pv[:, half * 512:(half + 1) * 512],
                                        hT[:, 2 * j:2 * j + 2,
                                           lt * 128:(lt + 1) * 128],
                                        wv[:, j, :, half * 512:(half + 1) * 512],
                                        start=(j == 0), stop=(j == 3),
                                        perf_mode=PM.DoubleRow)
                            nc.vector.scalar_tensor_tensor(
                                vv[:, lt // 2, lt % 2, :, 0:HD],
                                pv[:].rearrange("p (h c) -> p h c", c=HD),
                                1.0 / SW,
                                bvrep[:].rearrange("p (h c) -> p h c", c=HD),
                                op0=OP.mult, op1=OP.add)

                # ---- Phase C ----
                with tc.tile_pool(name="phC", bufs=1) as cpool2:
                    for i in range(NDT):
                        ptp = pspool.tile([128, 512], BF16, tag="trp",
                                          name="ptp", bufs=2)
                        pt3 = ptp[:].rearrange("p (a b) -> p a b", b=128)
                        for par in range(2):
                            h = 2 * i + par
                            p0 = 64 * par
                            acc = pspool.tile([128, NQT, HD + 1], F32,
                                              tag="acc", name="acc", bufs=2)
                            pms = []
                            for jj in range(NLT // 2):
                                ps = pspool.tile([128, 1024], F32, tag="mm",
                                                 name="ps", bufs=2)
                                for t in range(2):
                                    ktt = 2 * jj + t
                                    sl = slice(t * 512, (t + 1) * 512)
                                    nc.tensor.matmul(
                                        ps[:, sl],
                                        kT[p0:p0 + 64, i:i + 2,
                                           ktt * 128:(ktt + 1) * 128],
                                        qT[p0:p0 + 64, i, :, :],
                                        start=True, stop=False,
                                        perf_mode=PM.DoubleRow,
                                        skip_group_check=True)
                                    nc.tensor.matmul(
                                        ps[:, sl], iddr[:, h, :, :],
                                        tor[:, ktt:ktt + 2, :],
                                        start=False, stop=True,
                                        perf_mode=PM.DoubleRow,
                                        skip_group_check=True)
                                pm2 = cpool2.tile([128, 2, 512], FP8,
                                                  tag="pm2", name="pm2",
                                                  bufs=10)
                                if allmask:
                                    nc.scalar.activation(
                                        pm2[:].rearrange("p a b -> p (a b)"),
                                        ps[:], ACT.Exp, bias=eshc[:])
                                else:
                                    for t in range(2):
                                        nc.scalar.activation(
                                            pm2[:, t, :],
                                            ps[:, t * 512:(t + 1) * 512],
                                            ACT.Exp,
                                            bias=km[:, 2 * jj + t:
                                                    2 * jj + t + 1])
                                pms.append(pm2)
                            for jj in range(NLT // 2):
                                for qt in range(NQT):
                                    nc.tensor.matmul(
                                        acc[:, qt, :],
                                        pms[jj][:, :, qt * 128:(qt + 1) * 128],
                                        vv[:, jj, :, h, :],
                                        start=(jj == 0),
                                        stop=(jj == NLT // 2 - 1),
                                        perf_mode=PM.DoubleRow)
                            rs = cpool2.tile([128, NQT], F32, tag="rs",
                                             name="rs", bufs=4)
                            rc = cpool2.tile([128, NQT], F32, tag="rs",
                                             name="rc", bufs=4)
                            nc.vector.tensor_scalar(
                                rs[:], acc[:, :, HD], 1e-30, None, op0=OP.add)
                            nc.vector.reciprocal(rc[:], rs[:])
                            for qt in range(NQT):
                                asb = cpool2.tile([128, HD], BF16, tag="asb",
                                                  name="asb", bufs=4)
                                nc.vector.tensor_scalar(
                                    asb[:], acc[:, qt, 0:HD], rc[:, qt:qt + 1],
                                    SA, op0=OP.mult, op1=OP.mult)
                                nc.tensor.transpose(
                                    pt3[p0:p0 + 64, qt, :], asb[:], identb[:])
                        nc.vector.tensor_copy(outT[:, i, :], ptp[:])

            # ---- Phase D ----
            with tc.tile_pool(name="wtsDE", bufs=1) as wde:
                wo = wde.tile([128, 4, 2, D], FP8, tag="wo", name="wo")
                w1 = wde.tile([128, 8, 2, F], FP8, tag="w1", name="w1")
                w2 = wde.tile([128, 34, 2, D], FP8, tag="w2", name="w2")
                nc.sync.dma_start(wo[:], d_wo[:])
                nc.sync.dma_start(w1[:], d_w1[:])
                nc.sync.dma_start(w2[:], d_w2[:])

                with tc.tile_pool(name="phD", bufs=1) as dpool:
                    s1d = dpool.tile([128, NQT], F32, tag="s1d", name="s1d")
                    s2d = dpool.tile([128, NQT], F32, tag="s2d", name="s2d")
                    rstd2 = dpool.tile([128, NQT], F32, tag="rstd2",
                                       name="rstd2")
                    nmr2 = dpool.tile([128, NQT], F32, tag="nmr2", name="nmr2")
                    xres = dpool.tile([128, NQT, D], F32, tag="xresd",
                                      name="xresd")
                    nc.sync.dma_start(xres[:], d_xres[:])
                    for qt in range(NQT):
                        pd = pspool.tile([128, 1024], F32, tag="mm",
                                         name="pd", bufs=2)
                        for half in range(2):
                            for j in range(4):
                                nc.tensor.matmul(
                                    pd[:, half * 512:(half + 1) * 512],
                                    outT[:, 2 * j:2 * j + 2,
                                         qt * 128:(qt + 1) * 128],
                                    wo[:, j, :, half * 512:(half + 1) * 512],
                                    start=(j == 0), stop=(j == 3),
                                    perf_mode=PM.DoubleRow)
                        nc.vector.scalar_tensor_tensor(
                            x2[:, qt, :], pd[:], 1.0 / (SW * SA),
                            xres[:, qt, :], op0=OP.mult, op1=OP.add)
                        ln_stats(dpool, x2[:, qt, :], s1d[:, qt:qt + 1],
                                 s2d[:, qt:qt + 1])
                    ln_finalize(dpool, s1d[:], s2d[:], NQT, rstd2[:], nmr2[:],
                                "lnfD")
                    for qt in range(NQT):
                        ln_norm_transpose(dpool, x2[:, qt, :],
                                          rstd2[:, qt:qt + 1],
                                          nmr2[:, qt:qt + 1], h2T, qt * 128,
                                          ddstT=dh2T)

                # ---- Phase E ----
                with tc.tile_pool(name="phE", bufs=1) as epool:
                    aT = epool.tile([128, NFT + 2, QS], FP8, tag="aT",
                                    name="aT")
                    nc.gpsimd.memset(aT[:, NFT, :], SG)
                    nc.gpsimd.memset(aT[:, NFT + 1, :], 0.0)
                    for fp in range(NFT // 2):
                        pa = pspool.tile([128, 1024], F32, tag="mm",
                                         name="pa", bufs=2)
                        for half in range(2):
                            ft = 2 * fp + half
                            sl = slice(half * 512, (half + 1) * 512)
                            for j in range(12):
                                if j < 4:
                                    lhs = w1[:, j, :, ft * 128:(ft + 1) * 128]
                                    rhs = h2T[:, 2 * j:2 * j + 2, :]
                                elif j < 8:
                                    lhs = w1[:, j, :, ft * 128:(ft + 1) * 128]
                                    rhs = h2T[:, 2 * (j - 4):2 * (j - 4) + 2, :]
                                else:
                                    lhs = w1[:, j - 8, :,
                                             ft * 128:(ft + 1) * 128]
                                    rhs = dh2T[:, 2 * (j - 8):2 * (j - 8) + 2, :]
                                nc.tensor.matmul(
                                    pa[:, sl], lhs, rhs,
                                    start=(j == 0), stop=(j == 11),
                                    perf_mode=PM.DoubleRow)
                        for half in range(2):
                            ft = 2 * fp + half
                            gsc = epool.tile([128, 512], BF16, tag="gsc",
                                             name="gsc", bufs=3)
                            nc.scalar.activation(
                                gsc[:], pa[:, half * 512:(half + 1) * 512],
                                ACT.Gelu_apprx_tanh, bias=b1sb[:, ft:ft + 1],
                                scale=1.0 / SW)
                            nc.gpsimd.tensor_scalar(
                                aT[:, ft, :], gsc[:], SG, None, op0=OP.mult)
                    for qt in range(NQT):
                        py = pspool.tile([128, 1024], F32, tag="mm",
                                         name="py", bufs=2)
                        ysb = epool.tile([128, D], F32, tag="ysb", name="ysb",
                                         bufs=2)
                        for half in range(2):
                            for j in range(34):
                                nc.tensor.matmul(
                                    py[:, half * 512:(half + 1) * 512],
                                    aT[:, 2 * (j % 17):2 * (j % 17) + 2,
                                       qt * 128:(qt + 1) * 128],
                                    w2[:, j, :, half * 512:(half + 1) * 512],
                                    start=(j == 0), stop=(j == 33),
                                    perf_mode=PM.DoubleRow)
                        nc.vector.scalar_tensor_tensor(
                            ysb[:], py[:], 1.0 / (SW * SG), x2[:, qt, :],
                            op0=OP.mult, op1=OP.add)
                        nc.sync.dma_start(d_y[:, qt, :], ysb[:])

    nc.compile()
    return nc


def _gelu_tanh(x):
    x = x.astype(np.float64)
    return 0.5 * x * (1.0 + np.tanh(np.sqrt(2.0 / np.pi) * (x + 0.044715 * x ** 3)))


_F8 = ml_dtypes.float8_e4m3


def _dr_layout(w):
    """[Din, M] f32 -> [128, Din//256, 2, M] (no quantization)."""
    din, m = w.shape
    return w.reshape(din // 256, 2, 128, m).transpose(2, 0, 1, 3)


def _pack_dr(w):
    return np.ascontiguousarray(_dr_layout(w)).astype(_F8)


def _pack_dr_comp(w):
    """fp8 main + fp8 residual slots along the j axis."""
    q1 = w.astype(_F8)
    r = w - q1.astype(np.float32)
    main = _dr_layout(q1.astype(np.float32))
    resid = _dr_layout(r.astype(_F8).astype(np.float32))
    return np.ascontiguousarray(
        np.concatenate([main, resid], axis=1)).astype(_F8)


def _rowtile(a, n):
    m = a.shape[1]
    return np.ascontiguousarray(a.reshape(n, 128, m).transpose(1, 0, 2))


def kernel(x, torus_dist, time_emb, mask, ln1_g, ln1_b, Wqkv, Wout,
           torus_scale, ln2_g, ln2_b, W1, b1, W2, b2, Wt, bt):
    x = np.asarray(x, np.float32)
    torus_dist = np.asarray(torus_dist, np.float32)
    time_emb = np.asarray(time_emb, np.float32)
    mask = np.asarray(mask)
    Wqkv = np.asarray(Wqkv, np.float32)
    sc_arr = np.asarray(torus_scale, np.float32)

    allmask = bool(np.all(mask))
    key = f"nc_{allmask}"
    if key not in _CACHED:
        _CACHED[key] = _build(allmask=allmask)
    nc = _CACHED[key]

    bf = ml_dtypes.bfloat16

    tp = (_gelu_tanh(time_emb) @ np.asarray(Wt, np.float64)
          + np.asarray(bt, np.float64))          # [B, 2D]
    scale, shift = tp[:, :D], tp[:, D:]
    g_eff = (np.asarray(ln1_g, np.float64)[None, :] * (1.0 + scale))
    b_eff = (np.asarray(ln1_b, np.float64)[None, :] * (1.0 + scale) + shift)

    Wq_r = np.asarray(Wqkv[:, 0:D], np.float64) / np.sqrt(HD)
    Wk_r = np.asarray(Wqkv[:, D:2 * D], np.float64)
    Wv_r = np.asarray(Wqkv[:, 2 * D:3 * D], np.float64)
    W1_r = np.asarray(W1, np.float64)
    g2 = np.asarray(ln2_g, np.float64)
    b2ln = np.asarray(ln2_b, np.float64)

    wq_b, wk_b, wv_b, bq_b, bv_b = [], [], [], [], []
    for b_ in range(B):
        ge = g_eff[b_][:, None]
        be = b_eff[b_]
        wq_b.append(_pack_dr((ge * Wq_r * SW).astype(np.float32)))
        wk_b.append(_pack_dr((ge * Wk_r * SW).astype(np.float32)))
        wv_b.append(_pack_dr((ge * Wv_r * SW).astype(np.float32)))
        bq = (be @ Wq_r).astype(np.float32)
        bq_b.append(np.ascontiguousarray(bq.reshape(NDT, 128).T))
        bv = (be @ Wv_r).astype(np.float32)
        bv_b.append(np.ascontiguousarray(np.tile(bv[None, :], (128, 1))))
    wo_dr = _pack_dr((np.asarray(Wout, np.float64) * SW).astype(np.float32))
    w1_dr = _pack_dr_comp((g2[:, None] * W1_r * SW).astype(np.float32))
    w2_aug = np.zeros((F + 256, D), np.float32)
    w2_aug[:F] = (np.asarray(W2, np.float64) * SW).astype(np.float32)
    w2_aug[F] = (np.asarray(b2, np.float64) * SW * SG).astype(np.float32)
    w2_dr = _pack_dr_comp(w2_aug)
    b1sb_eff = (np.asarray(b1, np.float64) + b2ln @ W1_r).astype(np.float32)
    b1sb = np.ascontiguousarray(b1sb_eff.reshape(NFT, 128).T)

    idh = np.zeros((128, H, 2, 128), np.float32)
    sc_full = np.broadcast_to(sc_arr, (H,)).astype(np.float32)
    for h in range(H):
        np.fill_diagonal(idh[:, h, 0, :], sc_full[h])
    iddr = idh.astype(_F8)

    km_full = np.where(mask, 0.0, -88.0).astype(np.float32)   # [B, L]

    in_maps = []
    for c in range(8):
        b_, qs_ = c // NC_PER_B, c % NC_PER_B
        rows = slice(qs_ * QS, (qs_ + 1) * QS)
        xt = _rowtile(x[b_].astype(bf), NLT)
        xres = _rowtile(np.ascontiguousarray(x[b_, rows]), NQT
                        ).astype(np.float32)
        torT = torus_dist[0, rows, :].T.astype(np.float32)    # [L, QS]
        tor = np.zeros((128, NLT + 1, QS), _F8)
        tor[:, :NLT, :] = _rowtile(-torT, NLT).astype(_F8)
        kmt = np.ascontiguousarray(
            (km_full[b_] + ESH).reshape(NLT, 128).T)
        in_maps.append({
            "xt": xt, "xres": xres, "tor": tor, "iddr": iddr,
            "wq": wq_b[b_], "wk": wk_b[b_], "wv": wv_b[b_],
            "wo": wo_dr, "w1": w1_dr, "w2": w2_dr,
            "biasq": bq_b[b_], "b1sb": b1sb, "km": kmt,
            "bvrep": bv_b[b_],
        })

    import os
    trace = bool(int(os.environ.get("DENOISER_TRACE", "0")))
    res = run_bass_kernel_spmd(nc, in_maps, core_ids=list(range(8)), trace=trace)
    _CACHED["last_results"] = res

    out = np.empty((B, L, D), np.float32)
    for c in range(8):
        b_, qs_ = c // NC_PER_B, c % NC_PER_B
        y = res.results[c]["y"]
        out[b_, qs_ * QS:(qs_ + 1) * QS, :] = (
            y.transpose(1, 0, 2).reshape(QS, D))
    return out
